# revision 35
# baseline (speedup 1.0000x reference)
"""Multi-head causal attention (B=4, S=2048, D=1024, H=16) for 8 Trainium2 cores.

Sharding: core c = (batch b = c//2, head-group g = c%2). Each core computes,
for its batch and its 8 heads: QKV projections, causal softmax attention, and
a partial output projection (its heads' rows of Wo). Host sums the two
head-group partials per batch and adds the output bias.

Device layout notes:
 - Q^T/K^T are stored fp8e4m3 only, in a DoubleRow-folded layout; the score
   matmuls run in fp8 DoubleRow perf mode (2x contraction per pass, 0.5
   cycles/row).  The causal mask is accumulated into the score PSUM with one
   DoubleRow matmul per diagonal chunk: A (values 4*(r<k)) @ B (-240 step)
   = -960*cnt, which exp() flushes to 0.
 - Probabilities (exp output) and V are bf16; PV matmuls accumulate f32 in
   PSUM with an extra ones-column of V so each head's softmax denominator
   lands in PSUM row 64.
 - Phase 1 (projections) is emitted per s-tile and interleaved into the
   attention of earlier q-tiles (q-tile j only needs s-tiles <= j), hiding
   the Act engine's exp stream under projection matmuls.
 - Normalization: denominators are broadcast to partitions 0:64 either via a
   rank-1 ones matmul + DVE reciprocal (outer phase, low latency) or via the
   DMA-shift + gpsimd partition_broadcast path (inner phase, zero PE/PSUM
   cost).
"""

import sys

if "/opt/trn_rl_repo" not in sys.path:
    sys.path.insert(0, "/opt/trn_rl_repo")

import numpy as np

B, S, D = 4, 2048, 1024
H, DH = 16, 64
NCORES = 8
GH = H // 2            # heads per core
GW = GH * DH           # head-group width (512)
NP = GW // 128         # head pairs per core (4)
SM_SCALE = float(1.0 / np.sqrt(np.float32(D)))
MBIG = 240.0           # fp8e4m3 (ieee) max finite; mask A=4 -> -960*cnt


def build_mha_kernel(S_, D_, debug=False, debug_taps=False):
    import concourse.bass as bass  # noqa: F401
    import concourse.mybir as mybir
    import concourse.tile as tile
    from concourse import bacc

    f32 = mybir.dt.float32
    f32r = mybir.dt.float32r
    bf16 = mybir.dt.bfloat16
    f8 = mybir.dt.float8e4
    DR = mybir.MatmulPerfMode.DoubleRow

    KT = D_ // 128          # input-dim tiles
    NQT = S_ // 512         # q tiles == s tiles
    nc = bacc.Bacc("TRN2", target_bir_lowering=False, debug=debug)

    XT_d = nc.dram_tensor("XT", [D_, S_], bf16, kind="ExternalInput")
    WQ_d = nc.dram_tensor("WQ", [D_, GW], bf16, kind="ExternalInput")
    WK_d = nc.dram_tensor("WK", [D_, GW], bf16, kind="ExternalInput")
    WV_d = nc.dram_tensor("WV", [D_, GW], bf16, kind="ExternalInput")
    WO_d = nc.dram_tensor("WO", [GW, D_], bf16, kind="ExternalInput")
    AT_d = nc.dram_tensor("AT", [64, 2, 128], f8, kind="ExternalInput")
    BB_d = nc.dram_tensor("BB", [64, 2, 640], f8, kind="ExternalInput")
    OB_d = nc.dram_tensor("OB", [1, 64], f32, kind="ExternalInput")
    ON_d = nc.dram_tensor("ON", [128, S_ // 128 * GH], bf16,
                          kind="ExternalInput")
    Y_d = nc.dram_tensor("Y", [S_, D_], bf16, kind="ExternalOutput")

    Exp = mybir.ActivationFunctionType.Exp

    with tile.TileContext(nc) as tc:
        with tc.tile_pool(name="const", bufs=1) as const_pool, \
             tc.tile_pool(name="big", bufs=1) as big_pool, \
             tc.tile_pool(name="att", bufs=12) as att_pool, \
             tc.tile_pool(name="nrm", bufs=3) as nrm_pool, \
             tc.tile_pool(name="ctxp", bufs=2, space="PSUM") as ctx_pool:

            # ---- persistent tiles ----
            # QDR[h][32c+pp, i, s] = Q^T[dh, s] of head pair c, head h, with
            # dh enumerated as (pp, i) pairs (any consistent order works --
            # the score contraction is invariant to dh relabeling).
            QDR = big_pool.tile([128, 2, 2, S_], f8, name="qdr")
            KDR = big_pool.tile([128, 2, 2, S_], f8, name="kdr")
            NKC = S_ // 128
            V_t = big_pool.tile([128, NKC, GH * 65], bf16)  # V + ones col/head
            atri = const_pool.tile([64, 2, 128], f8)
            bbig = const_pool.tile([64, 2, 640], f8)
            onesb = const_pool.tile([65, 64], f32r)
            WO_t = const_pool.tile([128, NP, D_], bf16)

            STP = [None]        # current st psum pool (stA inner / stB outer)
            FAST_NORM = [False]  # outer phase uses the low-latency den path
            PH1PS = [None]      # projection psum: ps1 pool inner, stB ring
                                # (half of a "ps" tile) in the last window

            _psn = [0]

            def ps1_tile():
                _psn[0] += 1
                if PH1PS[0] is not None:
                    return PH1PS[0].tile([128, 512], f32, tag="ps1",
                                         name=f"ps1_{_psn[0]}")
                full = STP[0].tile([128, 1024], f32, tag="ps",
                                   name=f"ps1b_{_psn[0]}")
                return full[:, 0:512]

            # ---------- attention emission helpers ----------
            def emit_scores_exp(qt, c, kc):
                jp = kc - 4 * qt
                # trims are exact: fp8 has no min-moving-size penalty
                trim = min(128 * jp, 384) if jp >= 0 else 0
                qs = qt * 512
                stp = STP[0].tile([128, 1024], f32, tag="ps")
                st2 = stp.rearrange("p (i n) -> p i n", n=512)
                for i in (0, 1):
                    nc.tensor.matmul(
                        st2[:, i, trim:512],
                        lhsT=KDR[32 * c:32 * c + 32, i, :,
                                 kc * 128:(kc + 1) * 128],
                        rhs=QDR[32 * c:32 * c + 32, i, :,
                                qs + trim:qs + 512],
                        start=True, stop=(jp < 0),
                        perf_mode=DR, skip_group_check=True,
                        tile_position=(32 * c, 0))
                if jp >= 0:
                    for i in (0, 1):
                        nc.tensor.matmul(
                            st2[:, i, trim:512],
                            lhsT=atri,
                            rhs=bbig[:, :, 128:128 + 512 - trim],
                            start=False, stop=True,
                            perf_mode=DR, skip_group_check=True)
                pt = att_pool.tile([128, 2, 512], bf16, tag="pt")
                nc.scalar.activation(
                    pt[:, :, trim:512], st2[:, :, trim:512],
                    Exp, scale=SM_SCALE)
                return pt, trim

            def emit_pv(c, kc, nkc, ctx, pt, trim):
                for i in (0, 1):
                    h = 2 * c + i
                    nc.tensor.matmul(
                        ctx[0:65, i * 512 + trim:i * 512 + 512],
                        lhsT=V_t[:, kc, h * 65:(h + 1) * 65],
                        rhs=pt[:, i, trim:512],
                        start=(kc == 0), stop=(kc == nkc - 1),
                        skip_group_check=True)

            def emit_norm_a(qt, c, ctx):
                # den row (psum p64) -> sbuf on DVE; cheap, off PE
                den = nrm_pool.tile([65, 1024], f32r, tag="den", bufs=2)
                nc.vector.tensor_copy(out=den[64:65, :],
                                      in_=ctx[64:65, 0:1024])
                return den

            def emit_norm_b(qt, c, ctx, ctxn_c, den):
                if FAST_NORM[0]:
                    # broadcast den to p0:64 via a rank-1 matmul with a ones
                    # row, reciprocal on DVE (base-0), then scale ctx.
                    denb = STP[0].tile([128, 1024], f32, tag="ps",
                                       name=f"denb{qt}_{c}")
                    for i in (0, 1):
                        nc.tensor.matmul(
                            denb[0:64, i * 512:(i + 1) * 512],
                            lhsT=onesb[64:65, :],
                            rhs=den[64:65, i * 512:(i + 1) * 512],
                            start=True, stop=True, skip_group_check=True,
                            tile_position=(64, 0))
                    rden = nrm_pool.tile([64, 1024], f32, tag="rden", bufs=2)
                    nc.vector.reciprocal_approx_fast(out=rden,
                                                     in_=denb[0:64, :])
                    bca = rden[:, 0:512]
                    bcb = rden[:, 512:1024]
                else:
                    # DMA-shift + gpsimd broadcast: no PE / PSUM involvement
                    # (used while projections own PE and PSUM is tight)
                    den0 = nrm_pool.tile([1, 1024], f32, tag="den0", bufs=2)
                    nc.sync.dma_start(den0, den[64:65, :].bitcast(f32))
                    recip = nrm_pool.tile([1, 1024], f32, tag="recip", bufs=2)
                    nc.vector.reciprocal_approx_fast(out=recip, in_=den0)
                    bct = nrm_pool.tile([64, 1024], f32, tag="bct", bufs=2)
                    nc.gpsimd.partition_broadcast(bct[:, 0:512],
                                                  recip[0:1, 0:512])
                    nc.gpsimd.partition_broadcast(bct[:, 512:1024],
                                                  recip[0:1, 512:1024])
                    bca = bct[:, 0:512]
                    bcb = bct[:, 512:1024]
                nc.vector.tensor_mul(
                    ctxn_c[0:64, :], ctx[0:64, 0:512], bca)
                tmpb = nrm_pool.tile([64, 512], bf16, tag="tmpb", bufs=2)
                nc.vector.tensor_mul(tmpb, ctx[0:64, 512:1024], bcb)
                nc.sync.dma_start(ctxn_c[64:128, :], tmpb)

            def emit_oproj_ss(qt, ctxn, ss):
                NOUT = max(1, D_ // 512)
                OW = min(512, D_)
                yp = STP[0].tile([128, 1024], f32, tag="ps",
                                 name=f"yp{qt}_{ss}")
                for c in range(NP):
                    for n in range(NOUT):
                        nc.tensor.matmul(
                            yp[:, n * OW:(n + 1) * OW],
                            lhsT=ctxn[c][:, ss * 128:(ss + 1) * 128],
                            rhs=WO_t[:, c, n * OW:(n + 1) * OW],
                            start=(c == 0), stop=(c == NP - 1))
                ys = nrm_pool.tile([128, NOUT * OW], bf16, tag="ys", bufs=2)
                nc.vector.tensor_copy(out=ys, in_=yp[:, 0:NOUT * OW])
                nc.sync.dma_start(
                    Y_d[qt * 512 + ss * 128: qt * 512 + (ss + 1) * 128, :], ys)

            pending = []    # norms awaiting their _b half
            opq = []        # (qt, ctxn, ss) oproj pieces, popped outer-phase
            ctxn_of = {}

            def emit_pair(qt, pair, fill, pace=2):
                nkc = 4 * qt + 4
                cs = (2 * pair, 2 * pair + 1)
                ctxn = ctxn_of[qt]
                ctxt = {c: ctx_pool.tile([128, 1024], f32, tag="ctx",
                                         name=f"ctx{qt}_{c}")
                        for c in cs}
                ndef = len(pending)
                pvq = []
                for kc in range(nkc):
                    if kc < ndef:
                        emit_norm_b(*pending[kc])
                        if kc == ndef - 1:
                            del pending[:]
                    rnd = [(c, kc) + emit_scores_exp(qt, c, kc) for c in cs]
                    if kc >= ndef:
                        while len(pvq) > 2 * len(cs):
                            c2, k2, p2, t2 = pvq.pop(0)
                            emit_pv(c2, k2, nkc, ctxt[c2], p2, t2)
                        if FAST_NORM[0] and opq and not fill:
                            emit_oproj_ss(*opq.pop(0))
                    pvq += rnd
                    # splice in projection groups per round
                    for _ in range(pace):
                        if fill:
                            fill.pop(0)()
                for c2, k2, p2, t2 in pvq:
                    emit_pv(c2, k2, nkc, ctxt[c2], p2, t2)
                pending.extend((qt, c, ctxt[c], ctxn[c],
                                emit_norm_a(qt, c, ctxt[c])) for c in cs)

            def finish_qt(qt):
                opq.extend((qt, ctxn_of[qt], ss) for ss in range(4))

            # ---------- phase-1 emission helpers ----------
            ph1 = {}

            def make_ph1(xw_pool):
                WQ_t = xw_pool.tile([128, KT, GW], bf16, tag="wq")
                WK_t = xw_pool.tile([128, KT, GW], bf16, tag="wk")
                WV_t = xw_pool.tile([128, KT, GW], bf16, tag="wv")
                XT_r = XT_d.rearrange("(kt p) s -> p kt s", p=128)

                def ph1_dmas(st, with_w=False):
                    xt = xw_pool.tile([128, KT, 512], bf16, tag="xt", bufs=2,
                                      name=f"xt{st}")
                    kh = max(1, KT // 4)
                    sl = slice(st * 512, (st + 1) * 512)
                    wqr = WQ_d.rearrange("(kt p) n -> p kt n", p=128)
                    if with_w:
                        # small first slices so the first matmul group can
                        # start while the bulk streams in
                        nc.sync.dma_start(xt[:, 0:kh], XT_r[:, 0:kh, sl])
                        nc.sync.dma_start(WQ_t[:, 0:kh], wqr[:, 0:kh])
                        nc.sync.dma_start(xt[:, kh:KT], XT_r[:, kh:KT, sl])
                        nc.sync.dma_start(WQ_t[:, kh:KT], wqr[:, kh:KT])
                        for wt, wd in ((WK_t, WK_d), (WV_t, WV_d)):
                            nc.sync.dma_start(
                                wt, wd.rearrange("(kt p) n -> p kt n",
                                                 p=128))
                    else:
                        nc.sync.dma_start(xt[:, 0:kh], XT_r[:, 0:kh, sl])
                        nc.sync.dma_start(xt[:, kh:KT], XT_r[:, kh:KT, sl])
                    return xt

                def ph1_groups(st, xt):
                    sl = slice(st * 512, (st + 1) * 512)
                    qf = xw_pool.tile([128, NP, 512], f8, tag="qf8", bufs=1,
                                      name=f"qf{st}")
                    kf = xw_pool.tile([128, NP, 512], f8, tag="kf8", bufs=1,
                                      name=f"kf{st}")
                    thunks = []

                    def qk_group(c, wt, stage, drt):
                        psqk = ps1_tile()
                        for kt in range(KT):
                            nc.tensor.matmul(
                                psqk[:, :],
                                lhsT=wt[:, kt, c * 128:(c + 1) * 128],
                                rhs=xt[:, kt, :],
                                start=(kt == 0), stop=(kt == KT - 1))
                        # cast f32 psum -> fp8 (DVE: gpsimd
                        # cannot access PSUM on real hw)
                        nc.vector.tensor_copy(out=stage[:, c, :],
                                              in_=psqk[:, :])
                        # partition fold 128 -> (32, 2, 2): staging row
                        # r = 4pp+2h+i (host-permuted W cols) lands at
                        # DR[32c+pp, h, i, :]
                        nc.sync.dma_start(drt[32 * c:32 * c + 32, :, :, sl],
                                          stage[:, c, :])

                    def v_group(sc):
                        psv = ps1_tile()
                        for kt in range(KT):
                            nc.tensor.matmul(
                                psv[:, :],
                                lhsT=xt[:, kt, sc * 128:(sc + 1) * 128],
                                rhs=WV_t[:, kt, :],
                                start=(kt == 0), stop=(kt == KT - 1))
                        nc.vector.tensor_copy(
                            out=V_t[:, st * 4 + sc].rearrange(
                                "p (h e) -> p h e", e=65)[:, :, 0:64],
                            in_=psv[:, :].rearrange("p (h d) -> p h d", d=64))

                    for c in range(NP):
                        thunks.append(lambda c=c: qk_group(c, WQ_t, qf, QDR))
                    for c in range(NP):
                        thunks.append(lambda c=c: qk_group(c, WK_t, kf, KDR))
                    for c in range(NP):
                        thunks.append(lambda sc=c: v_group(sc))
                    return thunks

                ph1["dmas"] = ph1_dmas
                ph1["groups"] = ph1_groups

            # ---------- schedule ----------
            for qt in range(NQT):
                ctxn_of[qt] = [nrm_pool.tile([128, 512], bf16,
                                             tag=f"ctxn{c}", bufs=3,
                                             name=f"ctxn{c}_{qt}")
                               for c in range(NP)]

            with tc.tile_pool(name="xw", bufs=1) as xw_pool:
                make_ph1(xw_pool)
                with tc.tile_pool(name="ps1", bufs=2,
                                  space="PSUM") as ps1_pool, \
                     tc.tile_pool(name="stA", bufs=1, space="PSUM") as stA:
                    STP[0] = stA
                    PH1PS[0] = ps1_pool
                    xt = ph1["dmas"](0, with_w=True)
                    # constants (needed from attention onward)
                    nc.sync.dma_start(atri, AT_d[:])
                    nc.sync.dma_start(bbig, BB_d[:])
                    nc.sync.dma_start(onesb[64:65, :], OB_d[:].bitcast(f32r))
                    nc.sync.dma_start(
                        WO_t, WO_d.rearrange("(c p) n -> p c n", p=128))
                    nc.sync.dma_start(
                        V_t.rearrange("p kc (h e) -> p (kc h) e",
                                      e=65)[:, :, 64:65],
                        ON_d[:, :, None])
                    # s-tile 0 projections run serially (nothing to overlap)
                    for g in ph1["groups"](0, xt):
                        g()
                    # q-tile j's attention hides s-tile j+1's projections
                    fill = []
                    for qt in range(NQT - 2):
                        xt = ph1["dmas"](qt + 1)
                        fill += ph1["groups"](qt + 1, xt)
                        pace = 2 if qt == 0 else 1
                        emit_pair(qt, 0, fill, pace)
                        emit_pair(qt, 1, fill, pace)
                        finish_qt(qt)
                    for g in fill:
                        g()
                    xt_last = ph1["dmas"](NQT - 1)

                with tc.tile_pool(name="stB", bufs=2, space="PSUM") as stB:
                    # last s-tile's projections ride the outer "ps" ring
                    STP[0] = stB
                    PH1PS[0] = None
                    FAST_NORM[0] = True
                    fill = ph1["groups"](NQT - 1, xt_last)
                    qt = NQT - 2
                    emit_pair(qt, 0, fill, pace=1)
                    for g in fill:
                        g()
                    emit_pair(qt, 1, None)
                    finish_qt(qt)
                    qt = NQT - 1
                    emit_pair(qt, 0, None)
                    emit_pair(qt, 1, None)
                    finish_qt(qt)
                    for p in pending:
                        emit_norm_b(*p)
                    del pending[:]
                    for piece in opq:
                        emit_oproj_ss(*piece)

    nc.compile()
    return nc


_NC_CACHE = {}


def _get_nc():
    key = (S, D)
    if key not in _NC_CACHE:
        _NC_CACHE[key] = build_mha_kernel(S, D)
    return _NC_CACHE[key]


def make_consts(S_):
    import ml_dtypes

    f8 = ml_dtypes.float8_e4m3

    def fold(m):
        # [128, w] -> [64, 2, w] with row r = 64i+p at (p, i)
        return np.ascontiguousarray(
            m.reshape(2, 64, -1).transpose(1, 0, 2)).astype(f8)

    r = np.arange(128)
    at = fold(4.0 * (r[:, None] < r[None, :]))                 # A[r,k] = r < k
    bb = fold(np.where(r[:, None] >= np.arange(640)[None, :] - 128,
                       np.float32(-MBIG), np.float32(0.0)))
    on = np.ones((128, S_ // 128 * GH), dtype=ml_dtypes.bfloat16)
    ob = np.ones((1, 64), dtype=np.float32)
    return at, bb, on, ob


def permute_qk_cols(W):
    """Permute W's columns (per 128-col head-pair block) so projection psum
    row r holds head (r%4)//2, dh 32*(r%2) + r//4 -- the order the single
    remap DMA needs to fold rows into the DoubleRow layout."""
    W = np.asarray(W)
    r = np.arange(128)
    perm = 64 * ((r % 4) // 2) + 32 * (r % 2) + r // 4
    cols = (np.arange(W.shape[1]) // 128) * 128
    full = cols + perm[np.arange(W.shape[1]) % 128]
    return np.ascontiguousarray(W[:, full])


def shard_inputs(X, Wq, Wk, Wv, Wo):
    """Build the 8 per-core input maps from full inputs (weights/activations
    shipped bf16; attention runs fp8/bf16 with f32 accumulation)."""
    import ml_dtypes

    bf = ml_dtypes.bfloat16
    X = np.asarray(X, dtype=np.float32).astype(bf)
    Wq = np.asarray(Wq, dtype=np.float32).astype(bf)
    Wk = np.asarray(Wk, dtype=np.float32).astype(bf)
    Wv = np.asarray(Wv, dtype=np.float32).astype(bf)
    Wo = np.asarray(Wo, dtype=np.float32).astype(bf)
    at, bb, on, ob = make_consts(S)
    in_maps = []
    for c in range(NCORES):
        b, g = c // 2, c % 2
        in_maps.append({
            "XT": np.ascontiguousarray(X[b].T),
            "WQ": permute_qk_cols(Wq[:, g * GW:(g + 1) * GW]),
            "WK": permute_qk_cols(Wk[:, g * GW:(g + 1) * GW]),
            "WV": np.ascontiguousarray(Wv[:, g * GW:(g + 1) * GW]),
            "WO": np.ascontiguousarray(Wo[g * GW:(g + 1) * GW, :]),
            "AT": at, "BB": bb, "ON": on, "OB": ob,
        })
    return in_maps


def kernel(X, Wq, Wk, Wv, Wo, bo):
    from concourse.bass_utils import run_bass_kernel_spmd

    nc = _get_nc()
    in_maps = shard_inputs(X, Wq, Wk, Wv, Wo)
    res = run_bass_kernel_spmd(nc, in_maps, core_ids=list(range(NCORES)))
    bo = np.asarray(bo, dtype=np.float32)
    Y = np.empty((B, S, D), dtype=np.float32)
    for b in range(B):
        Y[b] = (res.results[2 * b]["Y"].astype(np.float32)
                + res.results[2 * b + 1]["Y"].astype(np.float32) + bo)
    return Y
